# revision 1
# baseline (speedup 1.0000x reference)
"""Trainium2 Bass kernel for nn_LlamaQAttention2 (AWQ int4 QKV+O projections,
RoPE, causal attention). 8-core head-parallel tensor parallelism:

  - each core owns 4 of the 32 heads: computes q,k,v for its heads from the
    full hidden states (int4-dequant QKV projection), runs RoPE + causal
    attention, multiplies by its 512-row slice of the dequantized O weight,
    and writes a full-size partial output.
  - the o_proj all-reduce is realized on the host as the gather step
    (sum of the 8 partials), followed by the layout transpose.

All matmuls run in fp16 operands with fp32 PSUM accumulation. Weight dequant
(w - z) * s runs on-device: nibble unpack via shift pairs (GPSIMD), zero/scale
rows broadcast across partitions via 0-stride DMA, fused into fp16 weights on
the vector engine. RoPE cos/sin tables are computed on-device from the
positions input using range-reduced ACT Sin.

Self-contained: hardcodes shapes for B=2, S=1024, H=4096, 32 heads.
"""

import math
import numpy as np
from contextlib import ExitStack

import concourse.bass as bass
import concourse.tile as tile
from concourse import bacc, mybir
from concourse.bass_utils import run_bass_kernel_spmd
from concourse.masks import make_identity

F32, F16, I32 = mybir.dt.float32, mybir.dt.float16, mybir.dt.int32
A = mybir.AluOpType
ACTF = mybir.ActivationFunctionType

B, S, H = 2, 1024, 4096
BS = B * S                      # 2048 flattened tokens
NH, HD = 32, 128                # heads, head dim
NC = 8                          # cores
HPC = NH // NC                  # 4 heads per core
QC = HPC * HD                   # 512 q (=k=v) columns per core
G = 32                          # quant groups (qkv), group size 128
GO = QC // 128                  # 4 o-proj groups per core
SCALING = HD ** -0.5
TWO_PI = 2.0 * math.pi
LN1E4 = math.log(10000.0)
AWQ_ORDER = (0, 4, 1, 5, 2, 6, 3, 7)   # nibble j of output col c*8+j sits at bit 4*AWQ_ORDER[j]

NCHUNK = 256                    # seq chunk for qkv matmul rhs
NSEQT = BS // 128               # 16 seq tiles of 128
KB = H // 128                   # 32 contraction blocks

_BUILT = None


def _emit(nc, aps):
    pos_ap = aps["pos"]
    x_ap = aps["x"]
    wq_ap = aps["wq_pack"]
    sq_ap = aps["qkv_sc"]
    zq_ap = aps["qkv_zs"]
    wo_ap = aps["wo_pack"]
    so_ap = aps["o_sc"]
    zo_ap = aps["o_zs"]
    out_ap = aps["out_t"]

    with ExitStack() as ctx:
        tc = aps["tc"]

        # ------------------------------------------------------------------
        # persistent pools
        # ------------------------------------------------------------------
        const = ctx.enter_context(tc.tile_pool(name="const", bufs=1))
        dram = ctx.enter_context(tc.tile_pool(name="dram", bufs=1, space="DRAM"))
        qkvp = ctx.enter_context(tc.tile_pool(name="qkvp", bufs=1))

        xTd = dram.tile([KB, 128, BS], F16)
        zq16d = dram.tile([G, 3 * QC], F16)
        sq16d = dram.tile([G, 3 * QC], F16)
        zo16d = dram.tile([GO, H], F16)
        so16d = dram.tile([GO, H], F16)
        posd = dram.tile([1, BS], F32)

        ident = const.tile([128, 128], F16)
        make_identity(nc, ident)
        # additive causal mask: 0 where col <= row, -1e9 above the diagonal
        addmask = const.tile([128, 128], F32)
        nc.vector.memset(addmask[:], 0.0)
        nc.gpsimd.affine_select(
            out=addmask[:], in_=addmask[:], compare_op=A.is_ge, fill=-1e9,
            base=0, pattern=[[-1, 128]], channel_multiplier=1)

        cos2 = const.tile([128, BS], F16)
        sin2 = const.tile([128, BS], F16)
        with tc.tile_pool(name="angprep", bufs=1) as ap_pool:
            # inv_freq per partition p: 10000^(-(p % 64)/64)
            pidx = ap_pool.tile([128, 1], I32)
            nc.gpsimd.iota(pidx[:], pattern=[[0, 1]], base=0, channel_multiplier=1)
            nc.vector.tensor_scalar(pidx[:], pidx[:], 63, None, A.bitwise_and)
            pf = ap_pool.tile([128, 1], F32)
            nc.vector.tensor_copy(pf[:], pidx[:])
            invfreq = ap_pool.tile([128, 1], F32)
            nc.scalar.activation(invfreq[:], pf[:], ACTF.Exp, scale=-LN1E4 / 64.0)

            # positions -> [1, BS] f32 in DRAM (for partition broadcast)
            posi = ap_pool.tile([1, 2 * BS], I32)
            nc.sync.dma_start(posi[:], pos_ap.rearrange("a b -> (a b)")[None, :])
            posf = ap_pool.tile([1, BS], F32)
            nc.vector.tensor_copy(posf[:], posi[:, ::2])
            nc.sync.dma_start(posd[:], posf[:])

            posb = ap_pool.tile([128, BS], F32)
            nc.sync.dma_start(posb[:], posd[:].to_broadcast([128, BS]))

            # angle tables: ang = pos * invfreq ; cos2/sin2 [128, BS] f16
            ang = ap_pool.tile([128, BS], F32)
            nc.vector.tensor_scalar(ang[:], posb[:], invfreq[:], 1.0 / TWO_PI,
                                    A.mult, A.mult)  # = ang / 2pi
            ftmp = ap_pool.tile([128, BS], F32)
            itmp = ap_pool.tile([128, BS], I32)
            gtmp = ap_pool.tile([128, BS], F32)

            def range_reduce_sin(dst, f_ap):
                # u = f - int(f); u -= (u > 0.5); sin(2*pi*u)
                # robust to truncate (sim) vs round-to-nearest (hw) converts
                nc.vector.tensor_copy(itmp[:], f_ap)
                nc.vector.tensor_copy(gtmp[:], itmp[:])
                nc.vector.tensor_tensor(gtmp[:], f_ap, gtmp[:], A.subtract)
                nc.vector.tensor_scalar(ftmp[:], gtmp[:], 0.5, None, A.is_gt)
                nc.vector.tensor_tensor(gtmp[:], gtmp[:], ftmp[:], A.subtract)
                nc.scalar.activation(dst, gtmp[:], ACTF.Sin, scale=TWO_PI)

            range_reduce_sin(sin2[:], ang[:])
            # cos: f + 0.25 then same
            nc.vector.tensor_scalar(ang[:], ang[:], 0.25, None, A.add)
            range_reduce_sin(cos2[:], ang[:])
            # sin2 rows 0:64 negated (for rope: out_lo = p_lo*cos - p_hi*sin)
            nc.vector.tensor_scalar(sin2[0:64, :], sin2[0:64, :], -1.0, None, A.mult)

        # ------------------------------------------------------------------
        # zero/scale rows -> fp16 in DRAM scratch
        # ------------------------------------------------------------------
        with tc.tile_pool(name="zsprep", bufs=1) as zs:
            sq32 = zs.tile([G, 3 * QC], F32)
            nc.sync.dma_start(sq32[:], sq_ap[:])
            sq16 = zs.tile([G, 3 * QC], F16)
            nc.vector.tensor_copy(sq16[:], sq32[:])
            nc.sync.dma_start(sq16d[:], sq16[:])

            zqp = zs.tile([G, 3 * QC // 8], I32)
            nc.sync.dma_start(zqp[:], zq_ap[:])
            zqn = zs.tile([G, 3 * QC], I32)
            zqv = zqn[:].rearrange("p (c j) -> p c j", j=8)
            for j in range(8):
                nc.vector.tensor_scalar(zqv[:, :, j], zqp[:], 4 * AWQ_ORDER[j], 0xF,
                                        A.logical_shift_right, A.bitwise_and)
            zq16 = zs.tile([G, 3 * QC], F16)
            nc.vector.tensor_copy(zq16[:], zqn[:])
            nc.sync.dma_start(zq16d[:], zq16[:])

            so32 = zs.tile([GO, H], F32)
            nc.sync.dma_start(so32[:], so_ap[:])
            so16 = zs.tile([GO, H], F16)
            nc.vector.tensor_copy(so16[:], so32[:])
            nc.sync.dma_start(so16d[:], so16[:])

            zop = zs.tile([GO, H // 8], I32)
            nc.sync.dma_start(zop[:], zo_ap[:])
            zon = zs.tile([GO, H], I32)
            zov = zon[:].rearrange("p (c j) -> p c j", j=8)
            for j in range(8):
                nc.vector.tensor_scalar(zov[:, :, j], zop[:], 4 * AWQ_ORDER[j], 0xF,
                                        A.logical_shift_right, A.bitwise_and)
            zo16 = zs.tile([GO, H], F16)
            nc.vector.tensor_copy(zo16[:], zon[:])
            nc.sync.dma_start(zo16d[:], zo16[:])

        # ------------------------------------------------------------------
        # phase 1: X -> fp16, PE-transpose into k-major X^T DRAM scratch
        # xTd[k] is [128, BS]: h-block k on partitions, all tokens on free
        # ------------------------------------------------------------------
        with ExitStack() as p1:
            xp = p1.enter_context(tc.tile_pool(name="xprep", bufs=2))
            xa = p1.enter_context(tc.tile_pool(name="xasm", bufs=2))
            tps = p1.enter_context(tc.tile_pool(name="tps", bufs=4, space="PSUM"))
            xasm = None
            for st in range(NSEQT):
                xs = xp.tile([128, H], F32, tag="xs", name="xs")
                nc.sync.dma_start(xs[:], x_ap[st * 128:(st + 1) * 128, :])
                x16t = xp.tile([128, H], F16, tag="x16t", name="x16t")
                nc.scalar.copy(x16t[:], xs[:])
                if st % 4 == 0:
                    xasm = [xa.tile([128, 512], F16, tag=f"xasm{k}", name=f"xasm{k}")
                            for k in range(KB)]
                for k in range(KB):
                    tp = tps.tile([128, 128], F16, tag="tp1", name="tp1")
                    nc.tensor.transpose(tp[:], x16t[:, k * 128:(k + 1) * 128], ident[:])
                    eng = nc.scalar if (k + st) % 2 else nc.vector
                    if eng is nc.scalar:
                        nc.scalar.copy(xasm[k][:, (st % 4) * 128:(st % 4 + 1) * 128], tp[:])
                    else:
                        nc.vector.tensor_copy(xasm[k][:, (st % 4) * 128:(st % 4 + 1) * 128], tp[:])
                if st % 4 == 3:
                    cc = (st // 4) * 512
                    for k in range(KB):
                        nc.sync.dma_start(xTd[k, :, cc:cc + 512], xasm[k][:])

        # q^T/k^T per head [128, BS] (d on partitions), v natural [128, QC] per seq tile
        qT = [qkvp.tile([128, BS], F16, tag=f"qT{h}", name=f"qT{h}") for h in range(HPC)]
        kT = [qkvp.tile([128, BS], F16, tag=f"kT{h}", name=f"kT{h}") for h in range(HPC)]
        vN = [qkvp.tile([128, QC], F16, tag=f"vN{t}", name=f"vN{t}") for t in range(NSEQT)]

        # ------------------------------------------------------------------
        # phase 2: dequant full W_qkv, compute q^T,k^T (rope) and v natural
        # ------------------------------------------------------------------
        def dequant_slice(pool, dst, pk_src, z_src, s_src, ncols, tag_suffix):
            """Dequant a 128-row x ncols slice into dst (fp16 weight AP)."""
            pk = pool.tile([128, ncols // 8], I32, tag=f"pk{tag_suffix}", name=f"pk{tag_suffix}")
            nc.sync.dma_start(pk[:], pk_src)
            zb = pool.tile([128, ncols], F16, tag=f"zb{tag_suffix}", name=f"zb{tag_suffix}")
            nc.sync.dma_start(zb[:], z_src.to_broadcast([128, ncols]))
            sb = pool.tile([128, ncols], F16, tag=f"sb{tag_suffix}", name=f"sb{tag_suffix}")
            nc.sync.dma_start(sb[:], s_src.to_broadcast([128, ncols]))
            nib = pool.tile([128, ncols], I32, tag=f"nib{tag_suffix}", name=f"nib{tag_suffix}")
            nibv = nib[:].rearrange("p (c j) -> p c j", j=8)
            for j in range(8):
                nc.vector.tensor_scalar(nibv[:, :, j], pk[:], 4 * AWQ_ORDER[j], 0xF,
                                        A.logical_shift_right, A.bitwise_and)
            t16 = pool.tile([128, ncols], F16, tag=f"t16{tag_suffix}", name=f"t16{tag_suffix}")
            nc.vector.tensor_tensor(t16[:], nib[:], zb[:], A.subtract)
            nc.vector.tensor_tensor(dst, t16[:], sb[:], A.mult)

        with ExitStack() as p2:
            wpool = p2.enter_context(tc.tile_pool(name="wqkv", bufs=1))
            dq = p2.enter_context(tc.tile_pool(name="dq", bufs=2))
            xtp = p2.enter_context(tc.tile_pool(name="xtp", bufs=2))
            rt = p2.enter_context(tc.tile_pool(name="ropetmp", bufs=3))
            psq = p2.enter_context(tc.tile_pool(name="psqk", bufs=4, space="PSUM"))
            psv = p2.enter_context(tc.tile_pool(name="psv", bufs=4, space="PSUM"))

            wqkv = []
            for k in range(KB):
                w = wpool.tile([128, 3 * QC], F16, tag=f"wqkv{k}", name=f"wqkv{k}")
                for sl in range(3):
                    dequant_slice(
                        dq, w[:, sl * 512:(sl + 1) * 512],
                        wq_ap[k * 128:(k + 1) * 128, sl * 64:(sl + 1) * 64],
                        zq16d[k:k + 1, sl * 512:(sl + 1) * 512],
                        sq16d[k:k + 1, sl * 512:(sl + 1) * 512],
                        512, "w")
                wqkv.append(w)

            if "dbg_w0" in aps:
                nc.sync.dma_start(aps["dbg_w0"][:], wqkv[0][:, 0:1024])
                nc.sync.dma_start(aps["dbg_w7"][:], wqkv[7][:, 0:1024])

            for n in range(BS // NCHUNK):
                ncol = slice(n * NCHUNK, (n + 1) * NCHUNK)
                xts = []
                for k in range(KB):
                    xt = xtp.tile([128, NCHUNK], F16, tag=f"xt{k}", name=f"xt{k}")
                    nc.sync.dma_start(xt[:], xTd[k, :, ncol])
                    xts.append(xt)
                for m in range(8):
                    ps = psq.tile([128, NCHUNK], F32, tag="psqk", name="psqk")
                    for k in range(KB):
                        nc.tensor.matmul(ps[:], wqkv[k][:, m * 128:(m + 1) * 128],
                                         xts[k][:], start=(k == 0), stop=(k == KB - 1))
                    # rope evacuation
                    dst = qT[m] if m < HPC else kT[m - HPC]
                    c1 = rt.tile([128, NCHUNK], F32, tag="c1", name="c1")
                    nc.vector.tensor_tensor(c1[:], ps[:], cos2[:, ncol], A.mult)
                    t2 = rt.tile([128, NCHUNK], F32, tag="t2", name="t2")
                    nc.vector.tensor_tensor(t2[0:64, :], ps[64:128, :],
                                            sin2[0:64, ncol], A.mult)
                    nc.vector.tensor_tensor(t2[64:128, :], ps[0:64, :],
                                            sin2[64:128, ncol], A.mult)
                    nc.vector.tensor_tensor(dst[:, ncol], c1[:], t2[:], A.add)
                # v natural for this chunk
                for sm in range(NCHUNK // 128):
                    st = (n * NCHUNK) // 128 + sm
                    ps = psv.tile([128, QC], F32, tag="psv", name="psv")
                    for k in range(KB):
                        nc.tensor.matmul(ps[:], xts[k][:, sm * 128:(sm + 1) * 128],
                                         wqkv[k][:, 2 * QC:3 * QC],
                                         start=(k == 0), stop=(k == KB - 1))
                    nc.scalar.copy(vN[st][:], ps[:])

        # ------------------------------------------------------------------
        # phase 4: attention per (batch, head)
        # ------------------------------------------------------------------
        atp = ctx.enter_context(tc.tile_pool(name="atp", bufs=1))
        attnT = [atp.tile([128, BS], F16, tag=f"attnT{h}", name=f"attnT{h}")
                 for h in range(HPC)]

        with ExitStack() as p4:
            sc_ps = p4.enter_context(tc.tile_pool(name="scps", bufs=2, space="PSUM"))
            tp_ps = p4.enter_context(tc.tile_pool(name="tpps", bufs=2, space="PSUM"))
            at_ps = p4.enter_context(tc.tile_pool(name="atps", bufs=2, space="PSUM"))
            smx = p4.enter_context(tc.tile_pool(name="smx", bufs=3))
            ptp = p4.enter_context(tc.tile_pool(name="ptp", bufs=2))

            for b in range(B):
                for h in range(HPC):
                    for c in range(2):  # sq chunks of 512 within batch
                        pts = [ptp.tile([128, 512], F16, tag=f"pt{sb}", name=f"pt{sb}")
                               for sb in range(4 * c + 4)]
                        # zero garbage columns in diagonal-range tiles
                        for sb in range(4 * c, 4 * c + 4):
                            j = sb - 4 * c
                            if j > 0:
                                nc.vector.memset(pts[sb][:, 0:j * 128], 0.0)
                        for qt in range(4 * c, 4 * c + 4):
                            ext = 128 * (qt + 1)
                            qcol = b * S + qt * 128
                            ps = sc_ps.tile([128, 1024], F32, tag="scores", name="scores")
                            for sl in range((ext + 511) // 512):
                                w = min(512, ext - sl * 512)
                                nc.tensor.matmul(
                                    ps[:, sl * 512:sl * 512 + w],
                                    qT[h][:, qcol:qcol + 128],
                                    kT[h][:, b * S + sl * 512:b * S + sl * 512 + w],
                                    start=True, stop=True)
                            nc.vector.tensor_tensor(ps[:, ext - 128:ext],
                                                    ps[:, ext - 128:ext],
                                                    addmask[:], A.add)
                            m = smx.tile([128, 1], F32, tag="rmax", name="rmax")
                            nc.vector.tensor_reduce(m[:], ps[:, 0:ext],
                                                    mybir.AxisListType.X, A.max)
                            negm = smx.tile([128, 1], F32, tag="negm", name="negm")
                            nc.vector.tensor_scalar(negm[:], m[:], -SCALING, None, A.mult)
                            probs = smx.tile([128, 1024], F16, tag="probs", name="probs")
                            for sl in range((ext + 511) // 512):
                                w = min(512, ext - sl * 512)
                                nc.scalar.activation(
                                    probs[:, sl * 512:sl * 512 + w],
                                    ps[:, sl * 512:sl * 512 + w],
                                    ACTF.Exp, bias=negm[:], scale=SCALING)
                            rsum = smx.tile([128, 1], F32, tag="rsum", name="rsum")
                            nc.vector.tensor_reduce(rsum[:], probs[:, 0:ext],
                                                    mybir.AxisListType.X, A.add)
                            rinv = smx.tile([128, 1], F32, tag="rinv", name="rinv")
                            nc.vector.reciprocal(rinv[:], rsum[:])
                            nc.vector.tensor_scalar(probs[:, 0:ext], probs[:, 0:ext],
                                                    rinv[:], None, A.mult)
                            # transpose computed 128-blocks into probs^T tiles
                            j = qt - 4 * c
                            for sb in range(qt + 1):
                                tp = tp_ps.tile([128, 128], F16, tag="tp", name="tp")
                                nc.tensor.transpose(
                                    tp[:], probs[:, sb * 128:(sb + 1) * 128], ident[:])
                                nc.scalar.copy(pts[sb][:, j * 128:(j + 1) * 128], tp[:])
                        # PV
                        aps_t = at_ps.tile([128, 512], F32, tag="atp", name="atp")
                        nblk = 4 * c + 4
                        for sb in range(nblk):
                            nc.tensor.matmul(
                                aps_t[:], vN[b * 8 + sb][:, h * 128:(h + 1) * 128],
                                pts[sb][:], start=(sb == 0), stop=(sb == nblk - 1))
                        nc.scalar.copy(
                            attnT[h][:, b * S + c * 512:b * S + (c + 1) * 512], aps_t[:])

        if "dbg_qT" in aps:
            for h in range(HPC):
                nc.sync.dma_start(aps["dbg_qT"][h * 128:(h + 1) * 128, :], qT[h][:])
                nc.sync.dma_start(aps["dbg_kT"][h * 128:(h + 1) * 128, :], kT[h][:])
                nc.sync.dma_start(aps["dbg_aT"][h * 128:(h + 1) * 128, :], attnT[h][:])
            for t in range(NSEQT):
                nc.sync.dma_start(aps["dbg_v"][t * 128:(t + 1) * 128, :], vN[t][:])

        # ------------------------------------------------------------------
        # phase 5: dequant W_o, O projection, write out^T
        # ------------------------------------------------------------------
        with ExitStack() as p5:
            wpool = p5.enter_context(tc.tile_pool(name="wo", bufs=1))
            dq = p5.enter_context(tc.tile_pool(name="dqo", bufs=2))
            pso = p5.enter_context(tc.tile_pool(name="pso", bufs=4, space="PSUM"))
            ost = p5.enter_context(tc.tile_pool(name="ost", bufs=4))

            wo = [wpool.tile([128, H], F16, tag=f"wo{kb}", name=f"wo{kb}")
                  for kb in range(GO)]
            for kb in range(GO):
                for sl in range(8):
                    dequant_slice(
                        dq, wo[kb][:, sl * 512:(sl + 1) * 512],
                        wo_ap[kb * 128:(kb + 1) * 128, sl * 64:(sl + 1) * 64],
                        zo16d[kb:kb + 1, sl * 512:(sl + 1) * 512],
                        so16d[kb:kb + 1, sl * 512:(sl + 1) * 512],
                        512, "o")

            for m in range(H // 128):
                for n in range(BS // 512):
                    ps = pso.tile([128, 512], F32, tag="pso", name="pso")
                    for kb in range(GO):
                        nc.tensor.matmul(ps[:], wo[kb][:, m * 128:(m + 1) * 128],
                                         attnT[kb][:, n * 512:(n + 1) * 512],
                                         start=(kb == 0), stop=(kb == GO - 1))
                    o = ost.tile([128, 512], F32, tag="ostage", name="ostage")
                    nc.scalar.copy(o[:], ps[:])
                    nc.sync.dma_start(
                        out_ap[m * 128:(m + 1) * 128, n * 512:(n + 1) * 512], o[:])


def _build(debug_taps=False):
    global _BUILT
    if _BUILT is not None and not debug_taps:
        return _BUILT
    nc = bacc.Bacc("TRN2", target_bir_lowering=False, debug=False, num_devices=NC)
    aps = {
        "pos": nc.dram_tensor("pos", [B, 2 * S], I32, kind="ExternalInput").ap(),
        "x": nc.dram_tensor("x", [BS, H], F32, kind="ExternalInput").ap(),
        "wq_pack": nc.dram_tensor("wq_pack", [H, 3 * QC // 8], I32,
                                  kind="ExternalInput").ap(),
        "qkv_sc": nc.dram_tensor("qkv_sc", [G, 3 * QC], F32,
                                 kind="ExternalInput").ap(),
        "qkv_zs": nc.dram_tensor("qkv_zs", [G, 3 * QC // 8], I32,
                                 kind="ExternalInput").ap(),
        "wo_pack": nc.dram_tensor("wo_pack", [QC, H // 8], I32,
                                  kind="ExternalInput").ap(),
        "o_sc": nc.dram_tensor("o_sc", [GO, H], F32, kind="ExternalInput").ap(),
        "o_zs": nc.dram_tensor("o_zs", [GO, H // 8], I32,
                               kind="ExternalInput").ap(),
        "out_t": nc.dram_tensor("out_t", [H, BS], F32, kind="ExternalOutput").ap(),
    }
    if debug_taps:
        aps["dbg_qT"] = nc.dram_tensor("dbg_qT", [QC, BS], F16, kind="ExternalOutput").ap()
        aps["dbg_kT"] = nc.dram_tensor("dbg_kT", [QC, BS], F16, kind="ExternalOutput").ap()
        aps["dbg_aT"] = nc.dram_tensor("dbg_aT", [QC, BS], F16, kind="ExternalOutput").ap()
        aps["dbg_v"] = nc.dram_tensor("dbg_v", [BS, QC], F16, kind="ExternalOutput").ap()
        aps["dbg_w0"] = nc.dram_tensor("dbg_w0", [128, 2 * QC], F16, kind="ExternalOutput").ap()
        aps["dbg_w7"] = nc.dram_tensor("dbg_w7", [128, 2 * QC], F16, kind="ExternalOutput").ap()
    with tile.TileContext(nc) as tc:
        aps["tc"] = tc
        _emit(nc, aps)
    nc.compile()
    if not debug_taps:
        _BUILT = nc
    return nc


def _in_maps(positions, hidden_states, qkv_qweight, qkv_qzeros, qkv_scales,
             o_qweight, o_qzeros, o_scales):
    pos = np.ascontiguousarray(np.asarray(positions, dtype=np.int64)).view(np.int32)
    pos = pos.reshape(B, 2 * S)
    x = np.ascontiguousarray(np.asarray(hidden_states, dtype=np.float32)).reshape(BS, H)
    qw = np.asarray(qkv_qweight)
    qz = np.asarray(qkv_qzeros)
    qs = np.asarray(qkv_scales, dtype=np.float32)
    ow = np.asarray(o_qweight)
    oz = np.asarray(o_qzeros)
    osc = np.asarray(o_scales, dtype=np.float32)

    maps = []
    for i in range(NC):
        pc = 64 * i           # packed col offset within q section
        uc = 512 * i          # unpacked col offset
        wq = np.concatenate([qw[:, pc:pc + 64],
                             qw[:, 512 + pc:512 + pc + 64],
                             qw[:, 1024 + pc:1024 + pc + 64]], axis=1)
        zq = np.concatenate([qz[:, pc:pc + 64],
                             qz[:, 512 + pc:512 + pc + 64],
                             qz[:, 1024 + pc:1024 + pc + 64]], axis=1)
        sq = np.concatenate([qs[:, uc:uc + 512],
                             qs[:, H + uc:H + uc + 512],
                             qs[:, 2 * H + uc:2 * H + uc + 512]], axis=1)
        maps.append({
            "pos": np.ascontiguousarray(pos),
            "x": x,
            "wq_pack": np.ascontiguousarray(wq, dtype=np.int32),
            "qkv_sc": np.ascontiguousarray(sq, dtype=np.float32),
            "qkv_zs": np.ascontiguousarray(zq, dtype=np.int32),
            "wo_pack": np.ascontiguousarray(ow[uc:uc + 512, :], dtype=np.int32),
            "o_sc": np.ascontiguousarray(osc[4 * i:4 * i + 4, :], dtype=np.float32),
            "o_zs": np.ascontiguousarray(oz[4 * i:4 * i + 4, :], dtype=np.int32),
        })
    return maps


def kernel(positions, hidden_states, qkv_qweight, qkv_qzeros, qkv_scales,
           o_qweight, o_qzeros, o_scales, _trace=False, **run_kwargs):
    nc = _build()
    maps = _in_maps(positions, hidden_states, qkv_qweight, qkv_qzeros, qkv_scales,
                    o_qweight, o_qzeros, o_scales)
    res = run_bass_kernel_spmd(nc, maps, core_ids=list(range(NC)),
                               trace=_trace, **run_kwargs)
    acc = np.zeros((H, BS), dtype=np.float32)
    for i in range(NC):
        acc += res.results[i]["out_t"]
    out = acc.T.reshape(B, S, H).astype(np.float32)
    if _trace:
        kernel.last_results = res
    return out

